# revision 12
# baseline (speedup 1.0000x reference)
"""Trainium2 Bass kernel for causal single-head attention with QKV projections.

Problem shape: B=4, S=4096, E=512, H=64 (fp32 inputs, causal mask).

Strategy (8 NeuronCores, data-parallel):
  - core j handles batch j%4; half j//4 of that batch's query rows.
    Half 0 = rows [0,1024)+[3072,4096), half 1 = rows [1024,3072) so the
    causal-triangle work is balanced across cores.
  - Host pre-transposes Q/K/V slabs to [E, S] layout and casts to bf16 so all
    device matmuls have the contraction dim on partitions (no on-chip input
    transposes needed).
  - On device: project Qt=[H,Sq], Kt=[H,S], v=[S,H] (fp32 biases added
    exactly via per-partition bias), then flash-style causal attention with
    scores kept transposed [k-part, q-free]:
       St = Kt_chunk^T @ Qt  -> exp fused into the PSUM->SBUF evacuation
       O^T (+denominator row) = [v | 1]^T @ P accumulated in PSUM
    No max-subtraction (scores are bounded, |s|<1 after 1/sqrt(E) scaling, so
    exp is safe; softmax is shift-invariant so the result matches reference).
  - Two query-block streams are interleaved and the St matmul runs one chunk
    ahead of the PV matmul so the PE never stalls on the ScalarE exp.
  - Per 512-query block: transpose O^T back via PE, normalize by the
    denominator row, DMA out fp32.
  - The two halves have different causal structure; a tc.If on partition_id
    selects the matching fully-unrolled variant.
"""

import sys

sys.path.insert(0, "/opt/trn_rl_repo")

import math

import numpy as np
import ml_dtypes

B, S, E, H = 4, 4096, 512, 64
N_CORES = 8
SQ = S // 2  # 2048 query rows per core
JBLK = 512  # query block size
NJ = SQ // JBLK  # 4 query blocks per core
KCH = 128  # key chunk size
JGLOBALS = [[0, 1, 6, 7], [2, 3, 4, 5]]  # global 512-row q-block ids per half
JPAIRS = [(0, 3), (1, 2)]  # local block pairs with equal total chunk counts
SCALE = 1.0 / math.sqrt(float(E))

BF16 = ml_dtypes.bfloat16

_CACHE = {}


def _build():
    import concourse.mybir as mybir
    from concourse import bacc, tile

    f32 = mybir.dt.float32
    bf16 = mybir.dt.bfloat16

    nc = bacc.Bacc(
        "TRN2", target_bir_lowering=False, debug=False, num_devices=N_CORES
    )

    qT = nc.dram_tensor("qT", [E, SQ], bf16, kind="ExternalInput")
    kT = nc.dram_tensor("kT", [E, S], bf16, kind="ExternalInput")
    vT = nc.dram_tensor("vT", [E, S], bf16, kind="ExternalInput")
    wT = nc.dram_tensor("wT", [3, E, 2 * H], bf16, kind="ExternalInput")
    bqkv = nc.dram_tensor("bqkv", [3, 2 * H, 1], f32, kind="ExternalInput")
    out = nc.dram_tensor("out", [SQ, H], f32, kind="ExternalOutput")

    # Embedded constants: causal block mask (allowed = k <= q) and identities.
    tril_np = np.triu(np.ones((KCH, KCH), np.float32)).astype(BF16)
    ident_np = np.eye(128, dtype=BF16)
    identf_np = np.eye(128, dtype=np.float32)
    trilc = nc.inline_tensor(tril_np, name="trilc")
    identbc = nc.inline_tensor(ident_np, name="identbc")
    identfc = nc.inline_tensor(identf_np, name="identfc")

    with tile.TileContext(nc) as tc:
        pid = nc.partition_id()
        with (
            tc.tile_pool(name="cpool", bufs=1) as cpool,
            tc.tile_pool(name="ipool", bufs=1) as ipool,
        ):
            # ---- constants ----
            w_sb = cpool.tile([128, 3, 4, 2 * H], bf16, name="w_sb")
            nc.sync.dma_start(
                w_sb[:], wT.ap().rearrange("m (c p) h -> p m c h", p=128)
            )
            b_sb = cpool.tile([2 * H, 3], f32, name="b_sb")
            nc.sync.dma_start(b_sb[:], bqkv.ap().rearrange("m h one -> h (m one)"))
            tril_sb = cpool.tile([KCH, KCH], bf16, name="tril_sb")
            nc.sync.dma_start(tril_sb[:], trilc.ap())
            identb_sb = cpool.tile([128, 128], bf16, name="identb_sb")
            nc.sync.dma_start(identb_sb[:], identbc.ap())
            identf_sb = cpool.tile([128, 128], f32, name="identf_sb")
            nc.sync.dma_start(identf_sb[:], identfc.ap())
            zbias = cpool.tile([128, 1], f32, name="zbias")
            nc.vector.memset(zbias[:], 0.0)

            # ---- input DMAs (transposed layout, e on partitions) ----
            # qT first (all attention needs Qt), then K/V in interleaved
            # S-quarters so projections and attention can chase the stream.
            qT_sb = [
                ipool.tile([128, SQ], bf16, name=f"qT{c}", tag=f"qT{c}")
                for c in range(4)
            ]
            kT_sb = [
                ipool.tile([128, 4, S // 4], bf16, name=f"kT{c}", tag=f"kT{c}")
                for c in range(4)
            ]
            vT_sb = [
                ipool.tile([128, 4, S // 4], bf16, name=f"vT{c}", tag=f"vT{c}")
                for c in range(4)
            ]
            def dma_kv(srcd, dst, qt):
                for c in range(4):
                    nc.sync.dma_start(
                        dst[c][:, qt, :],
                        srcd.ap()[
                            128 * c : 128 * (c + 1),
                            (S // 4) * qt : (S // 4) * (qt + 1),
                        ],
                    )

            # first key quarter before qT so the first score matmuls can
            # start as early as possible, then interleave K/V quarters
            dma_kv(kT, kT_sb, 0)
            for c in range(4):
                nc.sync.dma_start(qT_sb[c][:], qT.ap()[128 * c : 128 * (c + 1), :])
            dma_kv(vT, vT_sb, 0)
            for qt in range(1, 4):
                dma_kv(kT, kT_sb, qt)
                dma_kv(vT, vT_sb, qt)

            def body(jglobals, vtag):
                """Whole per-core pipeline for one causal-structure variant:
                projections interleaved with the longer stream-pair's
                attention rounds (round r needs exactly key chunk r, which
                projection block r//4 produces), then the shorter pair."""
                with (
                    tc.tile_pool(name=f"bpool{vtag}", bufs=1) as bpool,
                    tc.tile_pool(name=f"bps{vtag}", bufs=1, space="PSUM") as bps,
                ):
                    Qt = bpool.tile([2 * H, SQ], bf16, name=f"Qt{vtag}")
                    Kt = bpool.tile([2 * H, S], bf16, name=f"Kt{vtag}")
                    vTp = bpool.tile([2 * H, S], bf16, name=f"vTp{vtag}")
                    v_sb = bpool.tile(
                        [128, S // KCH, 128], bf16, name=f"v_sb{vtag}"
                    )
                    # ones column = softmax denominator row; zero padding so
                    # the [128,128] PV stationary adds nothing above row H
                    nc.vector.memset(v_sb[:, :, H : H + 1], 1.0)
                    nc.vector.memset(v_sb[:, :, H + 1 :], 0.0)

                    def proj_block(dst, srcs, m, blk):
                        ps = bps.tile(
                            [2 * H, 512], f32, name=f"pj{vtag}_{m}_{blk}",
                            tag="proj", bufs=1,
                        )
                        qt, off = divmod(512 * blk, S // 4)
                        for c in range(4):
                            nc.tensor.matmul(
                                ps[:],
                                w_sb[:, m, c, :],
                                srcs[c][:, qt, off : off + 512]
                                if len(srcs[c].shape) == 3
                                else srcs[c][:, 512 * blk : 512 * (blk + 1)],
                                start=(c == 0),
                                stop=(c == 3),
                            )
                        nc.vector.tensor_scalar_add(
                            dst[:, 512 * blk : 512 * (blk + 1)],
                            ps[:],
                            b_sb[:, m : m + 1],
                        )

                    def v_block(vb):
                        tps = bps.tile(
                            [128, 4, H], bf16, name=f"vt{vtag}_{vb}", tag="vtr",
                            bufs=1,
                        )
                        for t in range(4):
                            nc.tensor.transpose(
                                tps[:, t, :],
                                vTp[0:H, 512 * vb + 128 * t : 512 * vb + 128 * (t + 1)],
                                identb_sb[0:H, 0:H],
                            )
                        nc.vector.tensor_copy(
                            v_sb[:, 4 * vb : 4 * (vb + 1), 0:H], tps[:]
                        )

                    def chunk_geom(nk, ki):
                        d = ki - (nk - 4)  # >=0 for the 4 diagonal chunks
                        qlo = 0 if d < 0 else KCH * d
                        return d, qlo

                    def emit_st_pair(st8, pair, ki):
                        active = [x for x in pair if ki < st8[x]["nk"]]
                        st2 = bps.tile(
                            [128, 2 * JBLK], f32,
                            name=f"st{vtag}_{pair[0]}_{ki}", tag="st", bufs=2,
                        )
                        p2 = bpool.tile(
                            [128, 2 * JBLK], bf16,
                            name=f"p{vtag}_{pair[0]}_{ki}", tag="p", bufs=6,
                        )
                        diag = []
                        span = []
                        for x in active:
                            s = st8[x]
                            d, qlo = chunk_geom(s["nk"], ki)
                            off = JBLK * (x - pair[0])
                            nc.tensor.matmul(
                                st2[:, off + qlo : off + JBLK],
                                Kt[:, KCH * ki : KCH * (ki + 1)],
                                Qt[:, JBLK * s["jl"] + qlo : JBLK * (s["jl"] + 1)],
                                start=True,
                                stop=True,
                            )
                            span.append((off + qlo, off + JBLK))
                            if d >= 0:
                                diag.append(off + qlo)
                        lo, hi = span[0][0], span[-1][1]
                        nc.scalar.activation(
                            p2[:, lo:hi],
                            st2[:, lo:hi],
                            mybir.ActivationFunctionType.Exp,
                            bias=zbias[:],
                            scale=float(SCALE) / 2.0,
                        )
                        for off in diag:
                            nc.vector.tensor_mul(
                                p2[:, off : off + KCH], p2[:, off : off + KCH],
                                tril_sb[:],
                            )
                        return p2

                    def emit_pv(st8, pair, x, ki, p2):
                        s = st8[x]
                        d, qlo = chunk_geom(s["nk"], ki)
                        off = JBLK * (x - pair[0])
                        nc.tensor.matmul(
                            s["ot"][:, qlo:JBLK],
                            v_sb[:, ki, :],
                            p2[:, off + qlo : off + JBLK],
                            start=(ki == 0),
                            stop=(ki == s["nk"] - 1),
                        )

                    def epilogue(ot, jl):
                        otf = bpool.tile(
                            [H + 1, JBLK], f32, name=f"otf{vtag}_{jl}", tag="otf",
                            bufs=2,
                        )
                        nc.vector.tensor_copy(otf[:], ot[0 : H + 1, :])
                        ost = bpool.tile(
                            [128, 4, H], f32, name=f"ost{vtag}_{jl}", tag="ost",
                            bufs=2,
                        )
                        for t in range(4):
                            otr = bps.tile(
                                [128, H + 1], f32, name=f"otr{vtag}_{jl}_{t}",
                                tag="st", bufs=2,
                            )
                            nc.tensor.transpose(
                                otr[:],
                                otf[:, 128 * t : 128 * (t + 1)],
                                identf_sb[0 : H + 1, 0 : H + 1],
                            )
                            rec = bpool.tile(
                                [128, 1], f32, name=f"rec{vtag}_{jl}_{t}",
                                tag="rec", bufs=2,
                            )
                            nc.vector.reciprocal(rec[:], otr[:, H : H + 1])
                            nc.vector.tensor_scalar_mul(
                                ost[:, t, :], otr[:, 0:H], rec[:]
                            )
                        nc.sync.dma_start(
                            out.ap()[JBLK * jl : JBLK * (jl + 1), :].rearrange(
                                "(t p) h -> p t h", p=128
                            ),
                            ost[:],
                        )

                    st8 = {}
                    for jl in range(NJ):
                        jg = jglobals[jl]
                        st8[jl] = {"jl": jl, "jg": jg, "nk": 4 * (jg + 1)}

                    def pair_step(pair, pbuf, r):
                        rounds = max(st8[x]["nk"] for x in pair)
                        if r < rounds:
                            pbuf[r] = emit_st_pair(st8, pair, r)
                        ki = r - 2
                        if ki in pbuf:
                            for x in pair:
                                if ki < st8[x]["nk"]:
                                    emit_pv(st8, pair, x, ki, pbuf[ki])
                            del pbuf[ki]
                        for x in pair:
                            if r - 2 == st8[x]["nk"] - 1:
                                epilogue(st8[x]["ot"], x)

                    big, small = (2, 3), (0, 1)
                    for x in big:
                        st8[x]["ot"] = bps.tile(
                            [128, JBLK], f32, name=f"ot{vtag}_{x}", tag="ot",
                            bufs=2,
                        )
                    big_rounds = max(st8[x]["nk"] for x in big)
                    pbuf = {}
                    for blk in (2, 3, 0, 1):  # big pair's query blocks first
                        proj_block(Qt, qT_sb, 0, blk)
                    for b in range(S // 512):
                        proj_block(Kt, kT_sb, 1, b)
                        proj_block(vTp, vT_sb, 2, b)
                        v_block(b)
                        for r in range(4 * b, 4 * b + 4):
                            if r < big_rounds + 2:
                                pair_step(big, pbuf, r)
                    for r in range(S // KCH, big_rounds + 2):
                        pair_step(big, pbuf, r)
                    for x in small:
                        st8[x]["ot"] = bps.tile(
                            [128, JBLK], f32, name=f"ot{vtag}_{x}", tag="ot",
                            bufs=2,
                        )
                    small_rounds = max(st8[x]["nk"] for x in small)
                    pbuf = {}
                    for r in range(small_rounds + 2):
                        pair_step(small, pbuf, r)

            with tc.If(pid <= 3) as cmp:
                body(JGLOBALS[0], 0)
            with cmp.Else():
                body(JGLOBALS[1], 1)

    nc.compile()
    return nc


def _get_nc():
    if "nc" not in _CACHE:
        _CACHE["nc"] = _build()
    return _CACHE["nc"]


def _numpy_fallback(query, key, value, Wq, bq, Wk, bk, Wv, bv, mask):
    """Exact reference math in numpy; only used if the mask is not causal."""
    q = np.einsum("bse,he->bsh", query, Wq) + bq
    k = np.einsum("bse,he->bsh", key, Wk) + bk
    v = np.einsum("bse,he->bsh", value, Wv) + bv
    scores = np.einsum("bqh,bkh->bqk", q, k) / np.sqrt(np.float32(query.shape[-1]))
    scores = np.where(np.asarray(mask), scores, -np.inf)
    scores -= scores.max(axis=-1, keepdims=True)
    w = np.exp(scores)
    w /= w.sum(axis=-1, keepdims=True)
    return np.einsum("bqk,bkh->bqh", w, v).astype(np.float32)


def _half_rows(arr_s_first, half):
    """Select this half's query rows from an [S, ...] array."""
    if half == 0:
        return np.concatenate([arr_s_first[0 : S // 4], arr_s_first[3 * S // 4 : S]])
    return arr_s_first[S // 4 : 3 * S // 4]


def _prepare_in_maps(query, key, value, Wq, bq, Wk, bk, Wv, bv):
    # Weight columns (and biases) are duplicated into partitions 64..127 so
    # the score matmuls contract over the full 128 partitions (K=64 matmuls
    # never un-throttle the PE clock); scores double, the exp scale halves.
    wT1 = np.stack([Wq.T, Wk.T, Wv.T])
    wT = np.concatenate([wT1, wT1], axis=-1).astype(BF16)
    b1 = np.stack([bq, bk, bv]).reshape(3, H)
    bqkv = np.concatenate([b1, b1], axis=-1).reshape(3, 2 * H, 1).astype(np.float32)
    kT_b = [np.ascontiguousarray(key[b].T).astype(BF16) for b in range(B)]
    vT_b = [np.ascontiguousarray(value[b].T).astype(BF16) for b in range(B)]
    in_maps = []
    for j in range(N_CORES):
        b, half = j % B, j // B
        qslab = _half_rows(query[b], half)
        in_maps.append(
            {
                "qT": np.ascontiguousarray(qslab.T).astype(BF16),
                "kT": kT_b[b],
                "vT": vT_b[b],
                "wT": wT,
                "bqkv": bqkv,
            }
        )
    return in_maps


def _assemble(results):
    out = np.empty((B, S, H), np.float32)
    for j in range(N_CORES):
        b, half = j % B, j // B
        r = results[j]["out"]
        if half == 0:
            out[b, 0 : S // 4] = r[0 : S // 4]
            out[b, 3 * S // 4 : S] = r[S // 4 :]
        else:
            out[b, S // 4 : 3 * S // 4] = r
    return out


def run(query, key, value, Wq, bq, Wk, bk, Wv, bv, mask, trace=False, **trace_kwargs):
    from concourse.bass_utils import run_bass_kernel_spmd

    mask = np.asarray(mask)
    causal = mask.shape == (1, S, S) and bool(
        np.array_equal(mask[0], np.tril(np.ones((S, S), dtype=bool)))
    )
    if not causal:
        return _numpy_fallback(
            query, key, value, Wq, bq, Wk, bk, Wv, bv, mask
        ), None

    args = [np.asarray(a, np.float32) for a in (query, key, value, Wq, bq, Wk, bk, Wv, bv)]
    nc = _get_nc()
    in_maps = _prepare_in_maps(*args)
    res = run_bass_kernel_spmd(
        nc, in_maps, core_ids=list(range(N_CORES)), trace=trace, **trace_kwargs
    )
    return _assemble(res.results), res


def kernel(query, key, value, Wq, bq, Wk, bk, Wv, bv, mask):
    out, _ = run(query, key, value, Wq, bq, Wk, bk, Wv, bv, mask)
    return out


if __name__ == "__main__":
    rng = np.random.default_rng(0)
    query = rng.standard_normal((B, S, E)).astype(np.float32)
    key = rng.standard_normal((B, S, E)).astype(np.float32)
    value = rng.standard_normal((B, S, E)).astype(np.float32)
    Wq = (rng.standard_normal((H, E)) * 0.02).astype(np.float32)
    Wk = (rng.standard_normal((H, E)) * 0.02).astype(np.float32)
    Wv = (rng.standard_normal((H, E)) * 0.02).astype(np.float32)
    bq = np.zeros(H, np.float32)
    bk = np.zeros(H, np.float32)
    bv = np.zeros(H, np.float32)
    mask = np.tril(np.ones((1, S, S), dtype=bool))
    out = kernel(query, key, value, Wq, bq, Wk, bk, Wv, bv, mask)
    exp = _numpy_fallback(query, key, value, Wq, bq, Wk, bk, Wv, bv, mask)
    err = np.linalg.norm(out - exp) / np.linalg.norm(exp)
    print("self-check rel err:", err)


# revision 13
# speedup vs baseline: 1.0541x; 1.0541x over previous
"""Trainium2 Bass kernel for causal single-head attention with QKV projections.

Problem shape: B=4, S=4096, E=512, H=64 (fp32 inputs, causal mask).

Strategy (8 NeuronCores, data-parallel):
  - core j handles batch j%4; half j//4 of that batch's query rows.
    Half 0 = rows [0,1024)+[3072,4096), half 1 = rows [1024,3072) so the
    causal-triangle work is balanced across cores.
  - Host pre-transposes Q/K/V slabs to [E, S] layout and casts to bf16 so all
    device matmuls have the contraction dim on partitions (no on-chip input
    transposes needed).
  - On device: project Qt=[H,Sq], Kt=[H,S], v=[S,H] (fp32 biases added
    exactly via per-partition bias), then flash-style causal attention with
    scores kept transposed [k-part, q-free]:
       St = Kt_chunk^T @ Qt  -> exp fused into the PSUM->SBUF evacuation
       O^T (+denominator row) = [v | 1]^T @ P accumulated in PSUM
    No max-subtraction (scores are bounded, |s|<1 after 1/sqrt(E) scaling, so
    exp is safe; softmax is shift-invariant so the result matches reference).
  - Two query-block streams are interleaved and the St matmul runs one chunk
    ahead of the PV matmul so the PE never stalls on the ScalarE exp.
  - Per 512-query block: transpose O^T back via PE, normalize by the
    denominator row, DMA out fp32.
  - The two halves have different causal structure; a tc.If on partition_id
    selects the matching fully-unrolled variant.
"""

import sys

sys.path.insert(0, "/opt/trn_rl_repo")

import math

import numpy as np
import ml_dtypes

B, S, E, H = 4, 4096, 512, 64
N_CORES = 8
SQ = S // 2  # 2048 query rows per core
JBLK = 512  # query block size
NJ = SQ // JBLK  # 4 query blocks per core
KCH = 128  # key chunk size
JGLOBALS = [[0, 1, 6, 7], [2, 3, 4, 5]]  # global 512-row q-block ids per half
JPAIRS = [(0, 3), (1, 2)]  # local block pairs with equal total chunk counts
SCALE = 1.0 / math.sqrt(float(E))

BF16 = ml_dtypes.bfloat16

_CACHE = {}


def _build():
    import concourse.mybir as mybir
    from concourse import bacc, tile

    f32 = mybir.dt.float32
    bf16 = mybir.dt.bfloat16

    nc = bacc.Bacc(
        "TRN2", target_bir_lowering=False, debug=False, num_devices=N_CORES
    )

    qT = nc.dram_tensor("qT", [E, SQ], bf16, kind="ExternalInput")
    kT = nc.dram_tensor("kT", [E, S], bf16, kind="ExternalInput")
    vT = nc.dram_tensor("vT", [E, S], bf16, kind="ExternalInput")
    wT = nc.dram_tensor("wT", [3, E, 2 * H], bf16, kind="ExternalInput")
    bqkv = nc.dram_tensor("bqkv", [3, 2 * H, 1], f32, kind="ExternalInput")
    out = nc.dram_tensor("out", [SQ, H], f32, kind="ExternalOutput")

    # Embedded constants: causal block mask (allowed = k <= q) and identities.
    tril_np = np.triu(np.ones((KCH, KCH), np.float32)).astype(BF16)
    ident_np = np.eye(128, dtype=BF16)
    identf_np = np.eye(128, dtype=np.float32)
    trilc = nc.inline_tensor(tril_np, name="trilc")
    identbc = nc.inline_tensor(ident_np, name="identbc")
    identfc = nc.inline_tensor(identf_np, name="identfc")

    with tile.TileContext(nc) as tc:
        pid = nc.partition_id()
        with (
            tc.tile_pool(name="cpool", bufs=1) as cpool,
            tc.tile_pool(name="ipool", bufs=1) as ipool,
        ):
            # ---- constants ----
            w_sb = cpool.tile([128, 3, 4, 2 * H], bf16, name="w_sb")
            nc.sync.dma_start(
                w_sb[:], wT.ap().rearrange("m (c p) h -> p m c h", p=128)
            )
            b_sb = cpool.tile([2 * H, 3], f32, name="b_sb")
            nc.sync.dma_start(b_sb[:], bqkv.ap().rearrange("m h one -> h (m one)"))
            tril_sb = cpool.tile([KCH, KCH], bf16, name="tril_sb")
            nc.sync.dma_start(tril_sb[:], trilc.ap())
            identb_sb = cpool.tile([128, 128], bf16, name="identb_sb")
            nc.sync.dma_start(identb_sb[:], identbc.ap())
            identf_sb = cpool.tile([128, 128], f32, name="identf_sb")
            nc.sync.dma_start(identf_sb[:], identfc.ap())
            zbias = cpool.tile([128, 1], f32, name="zbias")
            nc.vector.memset(zbias[:], 0.0)

            # ---- input DMAs (transposed layout, e on partitions) ----
            # qT first (all attention needs Qt), then K/V in interleaved
            # S-quarters so projections and attention can chase the stream.
            qT_sb = [
                ipool.tile([128, SQ], bf16, name=f"qT{c}", tag=f"qT{c}")
                for c in range(4)
            ]
            kT_sb = [
                ipool.tile([128, 4, S // 4], bf16, name=f"kT{c}", tag=f"kT{c}")
                for c in range(4)
            ]
            vT_sb = [
                ipool.tile([128, 4, S // 4], bf16, name=f"vT{c}", tag=f"vT{c}")
                for c in range(4)
            ]
            def dma_kv(srcd, dst, qt):
                for c in range(4):
                    nc.sync.dma_start(
                        dst[c][:, qt, :],
                        srcd.ap()[
                            128 * c : 128 * (c + 1),
                            (S // 4) * qt : (S // 4) * (qt + 1),
                        ],
                    )

            for c in range(4):
                nc.sync.dma_start(qT_sb[c][:], qT.ap()[128 * c : 128 * (c + 1), :])
            for qt in range(4):
                dma_kv(kT, kT_sb, qt)
                dma_kv(vT, vT_sb, qt)

            def body(jglobals, vtag):
                """Whole per-core pipeline for one causal-structure variant:
                projections interleaved with the longer stream-pair's
                attention rounds (round r needs exactly key chunk r, which
                projection block r//4 produces), then the shorter pair."""
                with (
                    tc.tile_pool(name=f"bpool{vtag}", bufs=1) as bpool,
                    tc.tile_pool(name=f"bps{vtag}", bufs=1, space="PSUM") as bps,
                ):
                    Qt = bpool.tile([2 * H, SQ], bf16, name=f"Qt{vtag}")
                    Kt = bpool.tile([2 * H, S], bf16, name=f"Kt{vtag}")
                    vTp = bpool.tile([2 * H, S], bf16, name=f"vTp{vtag}")
                    v_sb = bpool.tile(
                        [128, S // KCH, 128], bf16, name=f"v_sb{vtag}"
                    )
                    # ones column = softmax denominator row; zero padding so
                    # the [128,128] PV stationary adds nothing above row H
                    nc.vector.memset(v_sb[:, :, H : H + 1], 1.0)
                    nc.vector.memset(v_sb[:, :, H + 1 :], 0.0)

                    def proj_block(dst, srcs, m, blk):
                        ps = bps.tile(
                            [2 * H, 512], f32, name=f"pj{vtag}_{m}_{blk}",
                            tag="proj", bufs=1,
                        )
                        qt, off = divmod(512 * blk, S // 4)
                        for c in range(4):
                            nc.tensor.matmul(
                                ps[:],
                                w_sb[:, m, c, :],
                                srcs[c][:, qt, off : off + 512]
                                if len(srcs[c].shape) == 3
                                else srcs[c][:, 512 * blk : 512 * (blk + 1)],
                                start=(c == 0),
                                stop=(c == 3),
                            )
                        nc.vector.tensor_scalar_add(
                            dst[:, 512 * blk : 512 * (blk + 1)],
                            ps[:],
                            b_sb[:, m : m + 1],
                        )

                    def v_block(vb):
                        tps = bps.tile(
                            [128, 4, H], bf16, name=f"vt{vtag}_{vb}", tag="vtr",
                            bufs=1,
                        )
                        for t in range(4):
                            nc.tensor.transpose(
                                tps[:, t, :],
                                vTp[0:H, 512 * vb + 128 * t : 512 * vb + 128 * (t + 1)],
                                identb_sb[0:H, 0:H],
                            )
                        nc.vector.tensor_copy(
                            v_sb[:, 4 * vb : 4 * (vb + 1), 0:H], tps[:]
                        )

                    def chunk_geom(nk, ki):
                        d = ki - (nk - 4)  # >=0 for the 4 diagonal chunks
                        qlo = 0 if d < 0 else KCH * d
                        return d, qlo

                    def emit_st_pair(st8, pair, ki):
                        active = [x for x in pair if ki < st8[x]["nk"]]
                        st2 = bps.tile(
                            [128, 2 * JBLK], f32,
                            name=f"st{vtag}_{pair[0]}_{ki}", tag="st", bufs=2,
                        )
                        p2 = bpool.tile(
                            [128, 2 * JBLK], bf16,
                            name=f"p{vtag}_{pair[0]}_{ki}", tag="p", bufs=6,
                        )
                        diag = []
                        span = []
                        for x in active:
                            s = st8[x]
                            d, qlo = chunk_geom(s["nk"], ki)
                            off = JBLK * (x - pair[0])
                            nc.tensor.matmul(
                                st2[:, off + qlo : off + JBLK],
                                Kt[:, KCH * ki : KCH * (ki + 1)],
                                Qt[:, JBLK * s["jl"] + qlo : JBLK * (s["jl"] + 1)],
                                start=True,
                                stop=True,
                            )
                            span.append((off + qlo, off + JBLK))
                            if d >= 0:
                                diag.append(off + qlo)
                        lo, hi = span[0][0], span[-1][1]
                        nc.scalar.activation(
                            p2[:, lo:hi],
                            st2[:, lo:hi],
                            mybir.ActivationFunctionType.Exp,
                            bias=zbias[:],
                            scale=float(SCALE) / 2.0,
                        )
                        for off in diag:
                            nc.vector.tensor_mul(
                                p2[:, off : off + KCH], p2[:, off : off + KCH],
                                tril_sb[:],
                            )
                        return p2

                    def emit_pv(st8, pair, x, ki, p2):
                        s = st8[x]
                        d, qlo = chunk_geom(s["nk"], ki)
                        off = JBLK * (x - pair[0])
                        nc.tensor.matmul(
                            s["ot"][:, qlo:JBLK],
                            v_sb[:, ki, :],
                            p2[:, off + qlo : off + JBLK],
                            start=(ki == 0),
                            stop=(ki == s["nk"] - 1),
                        )

                    def epilogue(ot, jl):
                        otf = bpool.tile(
                            [H + 1, JBLK], f32, name=f"otf{vtag}_{jl}", tag="otf",
                            bufs=2,
                        )
                        nc.vector.tensor_copy(otf[:], ot[0 : H + 1, :])
                        ost = bpool.tile(
                            [128, 4, H], f32, name=f"ost{vtag}_{jl}", tag="ost",
                            bufs=2,
                        )
                        for t in range(4):
                            otr = bps.tile(
                                [128, H + 1], f32, name=f"otr{vtag}_{jl}_{t}",
                                tag="st", bufs=2,
                            )
                            nc.tensor.transpose(
                                otr[:],
                                otf[:, 128 * t : 128 * (t + 1)],
                                identf_sb[0 : H + 1, 0 : H + 1],
                            )
                            rec = bpool.tile(
                                [128, 1], f32, name=f"rec{vtag}_{jl}_{t}",
                                tag="rec", bufs=2,
                            )
                            nc.vector.reciprocal(rec[:], otr[:, H : H + 1])
                            nc.vector.tensor_scalar_mul(
                                ost[:, t, :], otr[:, 0:H], rec[:]
                            )
                        nc.sync.dma_start(
                            out.ap()[JBLK * jl : JBLK * (jl + 1), :].rearrange(
                                "(t p) h -> p t h", p=128
                            ),
                            ost[:],
                        )

                    st8 = {}
                    for jl in range(NJ):
                        jg = jglobals[jl]
                        st8[jl] = {"jl": jl, "jg": jg, "nk": 4 * (jg + 1)}

                    def pair_step(pair, pbuf, r):
                        rounds = max(st8[x]["nk"] for x in pair)
                        if r < rounds:
                            pbuf[r] = emit_st_pair(st8, pair, r)
                        ki = r - 2
                        if ki in pbuf:
                            for x in pair:
                                if ki < st8[x]["nk"]:
                                    emit_pv(st8, pair, x, ki, pbuf[ki])
                            del pbuf[ki]
                        for x in pair:
                            if r - 2 == st8[x]["nk"] - 1:
                                epilogue(st8[x]["ot"], x)

                    big, small = (2, 3), (0, 1)
                    for x in big:
                        st8[x]["ot"] = bps.tile(
                            [128, JBLK], f32, name=f"ot{vtag}_{x}", tag="ot",
                            bufs=2,
                        )
                    big_rounds = max(st8[x]["nk"] for x in big)
                    pbuf = {}
                    for blk in (2, 3, 0, 1):  # big pair's query blocks first
                        proj_block(Qt, qT_sb, 0, blk)
                    for b in range(S // 512):
                        proj_block(Kt, kT_sb, 1, b)
                        proj_block(vTp, vT_sb, 2, b)
                        v_block(b)
                        for r in range(4 * b, 4 * b + 4):
                            if r < big_rounds + 2:
                                pair_step(big, pbuf, r)
                    for r in range(S // KCH, big_rounds + 2):
                        pair_step(big, pbuf, r)
                    for x in small:
                        st8[x]["ot"] = bps.tile(
                            [128, JBLK], f32, name=f"ot{vtag}_{x}", tag="ot",
                            bufs=2,
                        )
                    small_rounds = max(st8[x]["nk"] for x in small)
                    pbuf = {}
                    for r in range(small_rounds + 2):
                        pair_step(small, pbuf, r)

            with tc.If(pid <= 3) as cmp:
                body(JGLOBALS[0], 0)
            with cmp.Else():
                body(JGLOBALS[1], 1)

    nc.compile()
    return nc


def _get_nc():
    if "nc" not in _CACHE:
        _CACHE["nc"] = _build()
    return _CACHE["nc"]


def _numpy_fallback(query, key, value, Wq, bq, Wk, bk, Wv, bv, mask):
    """Exact reference math in numpy; only used if the mask is not causal."""
    q = np.einsum("bse,he->bsh", query, Wq) + bq
    k = np.einsum("bse,he->bsh", key, Wk) + bk
    v = np.einsum("bse,he->bsh", value, Wv) + bv
    scores = np.einsum("bqh,bkh->bqk", q, k) / np.sqrt(np.float32(query.shape[-1]))
    scores = np.where(np.asarray(mask), scores, -np.inf)
    scores -= scores.max(axis=-1, keepdims=True)
    w = np.exp(scores)
    w /= w.sum(axis=-1, keepdims=True)
    return np.einsum("bqk,bkh->bqh", w, v).astype(np.float32)


def _half_rows(arr_s_first, half):
    """Select this half's query rows from an [S, ...] array."""
    if half == 0:
        return np.concatenate([arr_s_first[0 : S // 4], arr_s_first[3 * S // 4 : S]])
    return arr_s_first[S // 4 : 3 * S // 4]


def _prepare_in_maps(query, key, value, Wq, bq, Wk, bk, Wv, bv):
    # Weight columns (and biases) are duplicated into partitions 64..127 so
    # the score matmuls contract over the full 128 partitions (K=64 matmuls
    # never un-throttle the PE clock); scores double, the exp scale halves.
    wT1 = np.stack([Wq.T, Wk.T, Wv.T])
    wT = np.concatenate([wT1, wT1], axis=-1).astype(BF16)
    b1 = np.stack([bq, bk, bv]).reshape(3, H)
    bqkv = np.concatenate([b1, b1], axis=-1).reshape(3, 2 * H, 1).astype(np.float32)
    kT_b = [np.ascontiguousarray(key[b].T).astype(BF16) for b in range(B)]
    vT_b = [np.ascontiguousarray(value[b].T).astype(BF16) for b in range(B)]
    in_maps = []
    for j in range(N_CORES):
        b, half = j % B, j // B
        qslab = _half_rows(query[b], half)
        in_maps.append(
            {
                "qT": np.ascontiguousarray(qslab.T).astype(BF16),
                "kT": kT_b[b],
                "vT": vT_b[b],
                "wT": wT,
                "bqkv": bqkv,
            }
        )
    return in_maps


def _assemble(results):
    out = np.empty((B, S, H), np.float32)
    for j in range(N_CORES):
        b, half = j % B, j // B
        r = results[j]["out"]
        if half == 0:
            out[b, 0 : S // 4] = r[0 : S // 4]
            out[b, 3 * S // 4 : S] = r[S // 4 :]
        else:
            out[b, S // 4 : 3 * S // 4] = r
    return out


def run(query, key, value, Wq, bq, Wk, bk, Wv, bv, mask, trace=False, **trace_kwargs):
    from concourse.bass_utils import run_bass_kernel_spmd

    mask = np.asarray(mask)
    causal = mask.shape == (1, S, S) and bool(
        np.array_equal(mask[0], np.tril(np.ones((S, S), dtype=bool)))
    )
    if not causal:
        return _numpy_fallback(
            query, key, value, Wq, bq, Wk, bk, Wv, bv, mask
        ), None

    args = [np.asarray(a, np.float32) for a in (query, key, value, Wq, bq, Wk, bk, Wv, bv)]
    nc = _get_nc()
    in_maps = _prepare_in_maps(*args)
    res = run_bass_kernel_spmd(
        nc, in_maps, core_ids=list(range(N_CORES)), trace=trace, **trace_kwargs
    )
    return _assemble(res.results), res


def kernel(query, key, value, Wq, bq, Wk, bk, Wv, bv, mask):
    out, _ = run(query, key, value, Wq, bq, Wk, bk, Wv, bv, mask)
    return out


if __name__ == "__main__":
    rng = np.random.default_rng(0)
    query = rng.standard_normal((B, S, E)).astype(np.float32)
    key = rng.standard_normal((B, S, E)).astype(np.float32)
    value = rng.standard_normal((B, S, E)).astype(np.float32)
    Wq = (rng.standard_normal((H, E)) * 0.02).astype(np.float32)
    Wk = (rng.standard_normal((H, E)) * 0.02).astype(np.float32)
    Wv = (rng.standard_normal((H, E)) * 0.02).astype(np.float32)
    bq = np.zeros(H, np.float32)
    bk = np.zeros(H, np.float32)
    bv = np.zeros(H, np.float32)
    mask = np.tril(np.ones((1, S, S), dtype=bool))
    out = kernel(query, key, value, Wq, bq, Wk, bk, Wv, bv, mask)
    exp = _numpy_fallback(query, key, value, Wq, bq, Wk, bk, Wv, bv, mask)
    err = np.linalg.norm(out - exp) / np.linalg.norm(exp)
    print("self-check rel err:", err)
